# revision 7
# baseline (speedup 1.0000x reference)
"""DiscriminativeLoss, single-core TRN2 Bass kernel, one fused NEFF.

The axon tunnel has ~84ms RTT per synchronization; host<->device bandwidth
is ~50MB/s. The whole game is ONE pipelined dispatch+fetch RPC:

  kernel(data fp32 [32,1024,1024] on dev, labels int32 [1024,1024] on dev)
    -> one jitted program (bass custom call, core 0 only)
    -> NEFF computes the ENTIRE scalar loss on device (~1ms)
    -> np.asarray(out [1,1]) = single await+fetch RPC

No XLA prep programs, no 8-core reshard, no collective, no host postprocess.

NEFF structure (single core, n = 1M pixels):
  lab pass: labels int32 -> fp16 DRAM scratch [1,H,W]
  phase A (8x4 tiles of [128h, 32d x 256w]):
    gpsimd cast-DMA fp32->fp16 in, ACT square, DVE d-tree -> x2 plane,
    onehot via stride-0 views + is_equal, 64 G=4-grouped matmuls into
    PSUM stats [128,132] (diag-folded to [C,33] at the end),
    fp16 scratch writes [34,H,W] (planes: x, ones, x2)
  center math: centers, chat [34,32] = {-2c^T, ||c||^2, 1}, pairwise
    dist term + reg term on [32,32] tiles
  phase B (512 quads of 2048 px = 4 col-tiled chunks of 512):
    xbh [34,2048] <- scratch; psD2 [128,512] = 4 col-strip matmuls
    (chunk g -> partitions 32g..32g+31); ACT sqrt -> d; ACT relu(d-1);
    stt1 mask = (lab==iota)*hd; stt2 accum mask^2 -> per-quad col;
    running accV [128,1]
  final: var = sum(accV)/C; dist = (sum hinge^2 - 288)/992; reg term;
    out [1,1] fp32 = var + dist + reg
"""
import os
import time
import numpy as np

D, H, W, C = 32, 1024, 1024, 32
N = H * W
SA = D + 1            # stats planes: x(32) + ones
DB = D + 2            # scratch planes: x(32) + ones + x2
WBLK = 256            # phase A w block
GEXP = WBLK // 4      # 64 matmuls per A tile
QCH = 512             # phase B chunk (psum bank)
DELTA_VAR, DELTA_DIST = 1.0, 2.0
SQRT_D = float(np.sqrt(np.float64(D)))
_DEBUG = bool(os.environ.get("KERNEL_DEBUG"))


def _log(msg):
    if _DEBUG:
        print(f"[kernel2] {msg}", flush=True)


def _numpy_ref(data, labels, cluster_ids):
    Cn = int(cluster_ids)
    data = np.asarray(data, np.float32)
    x = data.reshape(data.shape[0], -1).T.astype(np.float64)
    lab = np.asarray(labels).reshape(-1)
    counts = np.bincount(lab, minlength=Cn).astype(np.float64)
    sums = np.zeros((Cn, data.shape[0]))
    np.add.at(sums, lab, x)
    centers = sums / counts[:, None]
    d = np.linalg.norm(x - centers[lab], axis=1)
    var_term = np.sum(np.maximum(d - DELTA_VAR, 0.0) ** 2) / Cn
    diff = centers[:, None, :] - centers[None, :, :]
    sq = np.sum(diff * diff, axis=-1)
    eye = np.eye(Cn)
    cd = np.sqrt(sq + eye)
    hinge = np.maximum(2.0 * DELTA_DIST - cd, 0.0) ** 2 * (1.0 - eye)
    dist_term = np.sum(hinge) / (Cn * (Cn - 1))
    reg_term = np.sum(
        np.maximum(np.linalg.norm(centers, axis=1) - np.sqrt(data.shape[0]), 0.0)
    ) / Cn
    return np.float32(var_term + dist_term + reg_term)


def _build():
    import concourse.bacc as bacc
    import concourse.mybir as mybir
    import concourse.tile as tile
    from concourse.bass import ds

    dt16 = mybir.dt.float16
    dt32 = mybir.dt.float32
    dti = mybir.dt.int32
    ALU = mybir.AluOpType
    AF = mybir.ActivationFunctionType

    nc = bacc.Bacc("TRN2", target_bir_lowering=False, debug=False, num_devices=1)

    data_in = nc.dram_tensor("data", [D, H, W], dt32, kind="ExternalInput").ap()
    lab_in = nc.dram_tensor("labels", [H, W], dti, kind="ExternalInput").ap()
    iotap_in = nc.dram_tensor("iotap", [128, 1], dt32, kind="ExternalInput").ap()
    iotarow_in = nc.dram_tensor("iotarow", [1, C], dt16, kind="ExternalInput").ap()
    eye_in = nc.dram_tensor("eye", [C, C], dt32, kind="ExternalInput").ap()
    out = nc.dram_tensor("out", [1, 1], dt32, kind="ExternalOutput").ap()

    with tile.TileContext(nc) as tc:
        with (
            tc.tile_pool(name="st", bufs=1) as st,
            tc.tile_pool(name="pa", bufs=2) as pa,
            tc.tile_pool(name="pb", bufs=2) as pb,
            tc.tile_pool(name="psA", bufs=1, space="PSUM") as psA,
            tc.tile_pool(name="psB", bufs=2, space="PSUM") as psB,
            tc.tile_pool(name="psF", bufs=1, space="PSUM") as psF,
            tc.tile_pool(name="dram", bufs=1, space="DRAM") as dram,
        ):
            scr = dram.tile([DB, H, W], dt16)       # x planes + ones + x2
            lab16d = dram.tile([1, H, W], dt16)

            # ---- static constants
            iotaP = st.tile([128, 1], dt32)
            nc.sync.dma_start(iotaP[:], iotap_in[:, :])
            iotaRowT = st.tile([128, C], dt16)
            nc.sync.dma_start(iotaRowT[:], iotarow_in[0:1, :].broadcast_to([128, C]))
            eyeT = st.tile([C, C], dt32)
            nc.sync.dma_start(eyeT[:], eye_in[:, :])
            ones128 = st.tile([128, 1], dt32)
            nc.vector.memset(ones128[:], 1.0)
            onesRow = st.tile([1, C], dt32)
            nc.vector.memset(onesRow[:], 1.0)
            nbVar = st.tile([128, 1], dt32)
            nc.vector.memset(nbVar[:], -DELTA_VAR)

            # ---- labels int32 -> fp16 scratch
            for hb in range(8):
                labi = pa.tile([128, W], dti, tag="labi")
                nc.sync.dma_start(labi[:], lab_in[hb * 128:(hb + 1) * 128, :])
                labf = pa.tile([128, W], dt16, tag="labf")
                nc.vector.tensor_copy(labf[:], labi[:])
                nc.sync.dma_start(
                    lab16d[0:1, hb * 128:(hb + 1) * 128, :].rearrange(
                        "o h w -> (o h) w"
                    ),
                    labf[:],
                )

            # ---- phase A
            psStats = psA.tile([128, SA * 4], dt32)
            nc.vector.memset(psStats[:], 0.0)
            with tc.For_i(0, H, 128) as h0:
                with tc.For_i(0, W, WBLK) as w0:
                    xa = pa.tile([128, DB * WBLK], dt16, tag="xa")
                    xa3 = xa[:].rearrange("p (a w) -> p a w", a=DB)
                    nc.gpsimd.dma_start(
                        xa3[:, 0:D, :],
                        data_in[:, ds(h0, 128), ds(w0, WBLK)].rearrange(
                            "d h w -> h d w"
                        ),
                    )
                    nc.vector.memset(xa3[:, D, :], 1.0)
                    sq16 = pa.tile([128, D * WBLK], dt16, tag="sq16")
                    sq3 = sq16[:].rearrange("p (a w) -> p a w", a=D)
                    nc.scalar.activation(sq3[:, :, :], xa3[:, 0:D, :], AF.Square)
                    # d-tree reduce: 32 -> 16 -> 8 -> 4 -> 2 -> 1 (into x2 plane)
                    for half in (16, 8, 4, 2):
                        nc.vector.tensor_tensor(
                            sq3[:, 0:half, :],
                            sq3[:, 0:half, :],
                            sq3[:, half:2 * half, :],
                            ALU.add,
                        )
                    nc.vector.tensor_tensor(
                        xa3[:, D + 1, :], sq3[:, 0, :], sq3[:, 1, :], ALU.add
                    )
                    # onehot for all 256 columns via stride-0 views
                    labt = pa.tile([128, WBLK], dt16, tag="labt")
                    nc.sync.dma_start(
                        labt[:],
                        lab16d[0:1, ds(h0, 128), ds(w0, WBLK)].rearrange(
                            "o h w -> (o h) w"
                        ),
                    )
                    ohALL = pa.tile([128, WBLK * C], dt16, tag="ohALL")
                    nc.vector.tensor_tensor(
                        ohALL[:].rearrange("p (w c) -> p w c", c=C),
                        labt[:].unsqueeze(2).broadcast_to([128, WBLK, C]),
                        iotaRowT[:].unsqueeze(1).broadcast_to([128, WBLK, C]),
                        ALU.is_equal,
                    )
                    for g in range(GEXP):
                        nc.tensor.matmul(
                            psStats[:],
                            ohALL[:, g * 128:(g + 1) * 128],
                            xa3[:, 0:SA, g * 4:(g + 1) * 4],
                            start=False,
                            stop=False,
                            skip_group_check=True,
                        )
                    # one merged scratch write: x planes + ones + x2
                    nc.sync.dma_start(
                        scr[:, ds(h0, 128), ds(w0, WBLK)].rearrange(
                            "a h w -> h a w"
                        ),
                        xa3[:, :, :],
                    )

            # ---- fold stats + center math
            psv = psStats[:].rearrange("p (a w) -> p a w", w=4)
            stats = st.tile([C, SA], dt32)
            nc.vector.tensor_copy(stats[:], psv[0:C, :, 0])
            for g in range(1, 4):
                nc.vector.tensor_tensor(
                    stats[:], stats[:], psv[g * C:(g + 1) * C, :, g], ALU.add
                )
            recip = st.tile([C, 1], dt32)
            nc.vector.reciprocal(recip[:], stats[:, D:D + 1])
            centers = st.tile([C, D], dt32)
            nc.vector.tensor_scalar(centers[:], stats[:, 0:D], recip[:], None, ALU.mult)
            centersT = st.tile([D, C], dt32)
            nc.vector.transpose(centersT[:], centers[:])
            c2sq = st.tile([C, D], dt32)
            nc.vector.tensor_tensor(c2sq[:], centers[:], centers[:], ALU.mult)
            c2col = st.tile([C, 1], dt32)
            nc.vector.tensor_reduce(c2col[:], c2sq[:], mybir.AxisListType.X, ALU.add)
            chat = st.tile([DB, C], dt16)
            nc.vector.tensor_scalar(chat[0:D, :], centersT[:], -2.0, None, ALU.mult)
            c2tmp = st.tile([C, C], dt32)
            nc.vector.memset(c2tmp[:], 0.0)
            nc.vector.tensor_copy(c2tmp[:, 0:1], c2col[:])
            c2T = st.tile([C, C], dt32)
            nc.vector.transpose(c2T[:], c2tmp[:])
            nc.vector.tensor_copy(chat[D:D + 1, :], c2T[0:1, :])
            onesRowC = st.tile([1, C], dt16)
            nc.vector.memset(onesRowC[:], 1.0)
            nc.sync.dma_start(chat[D + 1:DB, :], onesRowC[:])

            # ---- phase B
            accV = st.tile([128, 1], dt32)
            nc.vector.memset(accV[:], 0.0)
            with tc.For_i(0, H, 8) as h8:
                xbh = pb.tile([DB, 8192], dt16, tag="xbh")
                nc.sync.dma_start(
                    xbh[:],
                    scr[:, ds(h8, 8), :].rearrange("a h w -> a (h w)"),
                )
                labb = pb.tile([128, 4 * QCH], dt16, tag="labb")
                labb3 = labb[:].rearrange("p (q w) -> p q w", q=4)
                for g in range(4):
                    dh, wh = g // 2, g % 2
                    nc.sync.dma_start(
                        labb3[g * C:(g + 1) * C, :, :],
                        lab16d[0:1, ds(h8 + dh, 4, 2), wh * QCH:(wh + 1) * QCH]
                        .rearrange("o h w -> o h w")
                        .broadcast_to([C, 4, QCH]),
                    )
                for q in range(4):
                    psD2 = psB.tile([128, QCH], dt32, tag="psD2")
                    for g in range(4):
                        dh, wh = g // 2, g % 2
                        nc.tensor.matmul(
                            psD2[g * C:(g + 1) * C, :],
                            chat[:],
                            xbh[:, (2 * q + dh) * W + wh * QCH:
                                (2 * q + dh) * W + (wh + 1) * QCH],
                            start=True,
                            stop=True,
                            tile_position=(0, g * C),
                        )
                    d16 = pb.tile([128, QCH], dt16, tag="d16")
                    nc.scalar.activation(d16[:], psD2[:], AF.Sqrt)
                    hd16 = pb.tile([128, QCH], dt16, tag="hd16")
                    nc.scalar.activation(hd16[:], d16[:], AF.Relu, bias=nbVar[:])
                    msk = pb.tile([128, QCH], dt16, tag="msk")
                    nc.vector.scalar_tensor_tensor(
                        msk[:], labb[:, q * QCH:(q + 1) * QCH], iotaP[:],
                        hd16[:], ALU.is_equal, ALU.mult
                    )
                    junk = pb.tile([128, QCH], dt16, tag="junk")
                    accQ = pb.tile([128, 1], dt32, tag="accQ")
                    nc.vector.scalar_tensor_tensor(
                        junk[:], msk[:], 1.0, msk[:], ALU.mult, ALU.mult,
                        accum_out=accQ[:],
                    )
                    nc.vector.tensor_tensor(accV[:], accV[:], accQ[:], ALU.add)

            # ---- final scalar assembly
            psFin = psF.tile([1, 1], dt32, tag="psFin")
            nc.tensor.matmul(psFin[:], accV[:], ones128[:], start=True, stop=True)
            psG = psF.tile([C, C], dt32, tag="psG")
            nc.tensor.matmul(psG[:], centersT[:], centersT[:], start=True, stop=True)
            psC2F = psF.tile([C, C], dt32, tag="psC2F")
            nc.tensor.matmul(
                psC2F[:], onesRow[:], c2T[0:1, :], start=True, stop=True
            )
            t1 = st.tile([C, C], dt32)
            nc.vector.tensor_scalar(t1[:], psG[:], -2.0, c2col[:], ALU.mult, ALU.add)
            nc.vector.tensor_tensor(t1[:], t1[:], psC2F[:], ALU.add)
            nc.vector.tensor_tensor(t1[:], t1[:], eyeT[:], ALU.add)
            cd = st.tile([C, C], dt32)
            nc.scalar.activation(cd[:], t1[:], AF.Sqrt)
            hng = st.tile([C, C], dt32)
            nc.vector.tensor_scalar(
                hng[:], cd[:], -1.0, 2.0 * DELTA_DIST, ALU.mult, ALU.add
            )
            nc.vector.tensor_scalar(hng[:], hng[:], 0.0, None, ALU.max)
            hjunk = st.tile([C, C], dt32)
            distcol = st.tile([C, 1], dt32)
            nc.vector.scalar_tensor_tensor(
                hjunk[:], hng[:], 1.0, hng[:], ALU.mult, ALU.mult,
                accum_out=distcol[:],
            )
            psDist = psF.tile([1, 1], dt32, tag="psDist")
            nc.tensor.matmul(
                psDist[:], distcol[:], ones128[0:C, :], start=True, stop=True
            )
            norms = st.tile([C, 1], dt32)
            nc.scalar.activation(norms[:], c2col[:], AF.Sqrt)
            rh = st.tile([C, 1], dt32)
            nc.vector.tensor_scalar(rh[:], norms[:], -SQRT_D, 0.0, ALU.add, ALU.max)
            psReg = psF.tile([1, 1], dt32, tag="psReg")
            nc.tensor.matmul(psReg[:], rh[:], ones128[0:C, :], start=True, stop=True)

            r1 = st.tile([1, 1], dt32)
            nc.vector.tensor_scalar(r1[:], psFin[:], 1.0 / C, None, ALU.mult)
            r2 = st.tile([1, 1], dt32)
            nc.vector.tensor_scalar(
                r2[:], psDist[:], -9.0 * C, 1.0 / (C * (C - 1)), ALU.add, ALU.mult
            )
            r3 = st.tile([1, 1], dt32)
            nc.vector.tensor_scalar(r3[:], psReg[:], 1.0 / C, None, ALU.mult)
            nc.vector.tensor_tensor(r1[:], r1[:], r2[:], ALU.add)
            nc.vector.tensor_tensor(r1[:], r1[:], r3[:], ALU.add)
            nc.sync.dma_start(out[:, :], r1[:])

    nc.compile()
    return nc


class _Runtime:
    pass


_RT = None
_SETUP_ERR = None


def _setup():
    import jax
    import concourse.mybir as mybir
    from concourse.bass2jax import (
        _bass_exec_p,
        install_neuronx_cc_hook,
        partition_id_tensor,
    )

    t0 = time.time()
    install_neuronx_cc_hook()
    nc = _build()
    _log(f"build+compile {time.time() - t0:.2f}s")

    partition_name = (
        nc.partition_id_tensor.name if nc.partition_id_tensor is not None else None
    )
    in_names, out_names, out_avals, zero_outs = [], [], [], []
    for alloc in nc.m.functions[0].allocations:
        if not isinstance(alloc, mybir.MemoryLocationSet):
            continue
        name = alloc.memorylocations[0].name
        if alloc.kind == "ExternalInput":
            if name != partition_name:
                in_names.append(name)
        elif alloc.kind == "ExternalOutput":
            out_names.append(name)
            shape = tuple(alloc.tensor_shape)
            dtype = mybir.dt.np(alloc.dtype)
            out_avals.append(jax.core.ShapedArray(shape, dtype))
            zero_outs.append(np.zeros(shape, dtype))
    n_params = len(in_names)
    n_outs = len(out_avals)
    all_in_names = list(in_names) + list(out_names)
    if partition_name is not None:
        all_in_names.append(partition_name)

    out_idx = out_names.index("out")

    def _body(*args):
        operands = list(args)
        if partition_name is not None:
            operands.append(partition_id_tensor())
        outs = _bass_exec_p.bind(
            *operands,
            out_avals=tuple(out_avals),
            in_names=tuple(all_in_names),
            out_names=tuple(out_names),
            lowering_input_output_aliases=(),
            sim_require_finite=True,
            sim_require_nnan=True,
            nc=nc,
        )
        return outs[out_idx].reshape(())

    # No donation: the NEFF "out" tensor binds to the custom-call RESULT
    # buffer (out_rename wins over in_rename in neuronx_cc_hook), the NEFF
    # writes every element of it, and the zeros operand is an unused param.
    # A persistent device-resident zeros array avoids a per-call host
    # buffer upload on the critical dispatch path.
    jf = jax.jit(_body, keep_unused=True)

    dev0 = jax.devices()[0]
    iotap_np = np.tile(np.arange(C, dtype=np.float32), 4).reshape(128, 1)
    iotarow_np = np.arange(C, dtype=np.float16).reshape(1, C)
    eye_np = np.eye(C, dtype=np.float32)
    const_dev = {
        "iotap": jax.device_put(iotap_np, dev0),
        "iotarow": jax.device_put(iotarow_np, dev0),
        "eye": jax.device_put(eye_np, dev0),
    }

    # pre-resolved argument template: positions of data/labels among params
    arg_template = []
    data_pos = lab_pos = None
    for i, n in enumerate(in_names):
        if n == "data":
            data_pos = i
            arg_template.append(None)
        elif n == "labels":
            lab_pos = i
            arg_template.append(None)
        else:
            arg_template.append(const_dev[n])
    persist_zeros = [jax.device_put(z, dev0) for z in zero_outs]
    jax.block_until_ready(persist_zeros)
    arg_template.extend(persist_zeros)

    def _call_lazy(data, labels):
        args = list(arg_template)
        args[data_pos] = data
        args[lab_pos] = labels
        return jf(*args)

    def _call(data, labels):
        return np.asarray(_call_lazy(data, labels)).reshape(1, 1)

    # ---- warm with EXACT harness avals: rng outputs, uncommitted on dev0
    t0 = time.time()
    key = jax.random.key(0)
    k1, k2 = jax.random.split(key)
    wdata = jax.random.normal(k1, (D, H, W), dtype=np.float32)
    wlab = jax.random.randint(k2, (H, W), 0, C, dtype=np.int32)
    r1 = _call(wdata, wlab)
    _log(f"warm1 (rng avals) {time.time() - t0:.2f}s -> {r1[0, 0]:.4f}")
    # committed-device and host-numpy variants
    t0 = time.time()
    cdata = jax.device_put(np.zeros((D, H, W), np.float32), dev0)
    clab = jax.device_put(np.zeros((H, W), np.int32), dev0)
    jax.block_until_ready((cdata, clab))
    _call(cdata, clab)
    _log(f"warm2 (committed avals) {time.time() - t0:.2f}s")
    # timed rehearsal on rng inputs
    t0 = time.time()
    wdata2 = jax.random.normal(k1, (D, H, W), dtype=np.float32)
    wlab2 = jax.random.randint(k2, (H, W), 0, C, dtype=np.int32)
    jax.block_until_ready((wdata2, wlab2))
    t0 = time.time()
    r3 = _call(wdata2, wlab2)
    _log(f"rehearsal exec+fetch {time.time() - t0:.3f}s -> {r3[0, 0]:.4f}")
    # prime the pjit fastpath for the first real call
    for _ in range(2):
        t0 = time.time()
        rr = _call_lazy(wdata2, wlab2)
        _log(f"lazy prime dispatch {time.time() - t0:.4f}s")
    np.asarray(rr)

    rt = _Runtime()
    rt.call = _call
    rt.call_lazy = _call_lazy
    rt.expected_val = float(r3[0, 0])
    return rt


try:
    _RT = _setup()
except Exception as _e:  # noqa: BLE001
    import traceback

    traceback.print_exc()
    _SETUP_ERR = _e
    _RT = None


def kernel(data, labels, cluster_ids):
    if _RT is None:
        return _numpy_ref(data, labels, cluster_ids)
    try:
        if (
            int(cluster_ids) != C
            or tuple(data.shape) != (D, H, W)
            or tuple(labels.shape) != (H, W)
        ):
            return _numpy_ref(data, labels, cluster_ids)
        t0 = time.time()
        r = _RT.call_lazy(data, labels)
        _log(f"dispatch {time.time() - t0:.3f}s")
        return r
    except Exception as e:  # noqa: BLE001
        import traceback

        traceback.print_exc()
        print("BASS KERNEL2 FAILED; falling back to host compute:", e)
        return _numpy_ref(data, labels, cluster_ids)
